# revision 29
# baseline (speedup 1.0000x reference)
"""Causal multi-head attention kernel for 8 Trainium2 NeuronCores.

Problem: x(4,2048,512) -> qkv proj -> 8-head causal attention -> out proj.
Sharding: core c handles batch b=c//2, heads 4*(c%2)..4*(c%2)+3.
Each core returns a partial (2048,512) output (its 4 heads' contribution
through w_out); host sums the two cores of each batch and adds the
effective output bias (b_out + b_v @ w_out -- the V bias commutes through
the softmax-weighted average exactly, so it folds into the output bias).

Per-core device algorithm (bf16 matmuls, fp32 psum/softmax), fully
software-pipelined so the PE array never idles (idle gaps drop the PE
p-state from 2.4 GHz to 1.2 GHz for ~3us):

  P1a  QT/KT (128 = 2 heads x 64 hd, 512-query chunks) from
       host-pretransposed xT; Q/K biases added per-partition during the
       PSUM->SBUF evacuation via tensor_scalar_add (no rank-1 matmuls).
  P1b  V stored natural (keys on partitions) with a ones column per head
       so the PV matmul also produces softmax denominators.
  P2   per head pair: S_T = K Q^T (keys on partitions, queries free) in
       fp32 PSUM, causal diag masked by accumulating -1e5*(k>q) via a
       bf16 matmul, exp via ACT (scale=1/8 folded, no max subtraction --
       scores are O(10)), then [V|1]^T @ P^T accumulated in PSUM.
       Denominators: copy -> DMA-gather to 128 lanes -> DVE reciprocal ->
       DMA back to a row -> partition-broadcast via ones(1,64) matmul ->
       elementwise normalize.
  P3   output projection per 128-token chunk, DMA'd out immediately.

  Emission interleaves P1b/P1a(pair1)/normalize/P3 chunks into the P2
  kk-loops as PE fillers, so the exp stream (ACT engine, the secondary
  bottleneck) pipelines back-to-back under continuous PE work.
"""

import sys

import numpy as np

if "/opt/trn_rl_repo" not in sys.path:
    sys.path.insert(0, "/opt/trn_rl_repo")

import ml_dtypes

import concourse.bass as bass
import concourse.mybir as mybir
import concourse.tile as tile
from concourse import bacc
from concourse.bass_utils import run_bass_kernel_spmd

F32 = mybir.dt.float32
BF16 = mybir.dt.bfloat16
AF = mybir.ActivationFunctionType

S = 2048
D = 512
HD = 64
HPC = 4          # heads per core
NCORES = 8
SCALE = 0.125    # 1/sqrt(64)
VW = HD + 1      # 65: V plus ones column
VWS = HPC * VW   # 260

# column offsets inside the packed bf16 (128, FTOT) input
OFF_WQ = 0                      # 4 x 256
OFF_WK = OFF_WQ + 1024          # 4 x 256
OFF_XT = OFF_WK + 1024          # 4 x 2048
OFF_WVA = OFF_XT + 4 * S        # 4 x 260
OFF_WO = OFF_WVA + 4 * VWS      # 2 x 512
OFF_ONES = OFF_WO + 2 * D       # 128 (row 0 ones)
OFF_MASK = OFF_ONES + 128       # (128,128) 0/1 causal keep-mask (k<=q)
OFF_VONES = OFF_MASK + 128      # (128,260) ones-column marks for vaug
FTOT = OFF_VONES + VWS


def build_nc():
    nc = bacc.Bacc("TRN2", target_bir_lowering=False, debug=False)

    wpack = nc.dram_tensor("wpack", [128, FTOT], BF16,
                           kind="ExternalInput").ap()
    bcol = nc.dram_tensor("bcol", [128, 4], F32, kind="ExternalInput").ap()
    out = nc.dram_tensor("out", [S, D], F32, kind="ExternalOutput").ap()

    with tile.TileContext(nc) as tc:
        _build_kernel(tc, wpack, bcol, out)
    nc.compile()
    return nc


def _build_kernel(tc, wpack, bcol, out):
    nc = tc.nc
    from contextlib import ExitStack

    ctx = ExitStack()
    with ctx:
        pers = ctx.enter_context(tc.tile_pool(name="pers", bufs=1))
        spsum = ctx.enter_context(
            tc.tile_pool(name="spsum", bufs=2, space="PSUM"))   # scores
        opsum = ctx.enter_context(
            tc.tile_pool(name="opsum", bufs=1, space="PSUM"))   # PV accum
        p1p = ctx.enter_context(
            tc.tile_pool(name="p1p", bufs=2, space="PSUM"))     # P1/P3/bcast
        ptp = ctx.enter_context(tc.tile_pool(name="ptp", bufs=4))
        otp = ctx.enter_context(tc.tile_pool(name="otp", bufs=4))
        outp = ctx.enter_context(tc.tile_pool(name="outp", bufs=3))
        dnp = ctx.enter_context(tc.tile_pool(name="dnp", bufs=3))

        # ---------- input tiles + DMAs, compute-start order ----------
        w_qb = pers.tile([128, 1024], BF16, tag="w_qb", name="w_qb")
        w_bc = pers.tile([128, 4], F32, tag="w_bc", name="w_bc")
        w_k = pers.tile([128, 1024], BF16, tag="w_k", name="w_k")
        w_misc = pers.tile([128, 516], BF16, tag="w_misc", name="w_misc")
        w_wva = pers.tile([128, 1040], BF16, tag="w_wva", name="w_wva")
        w_wo = pers.tile([128, 1024], BF16, tag="w_wo", name="w_wo")
        x_t = [[pers.tile([128, 512], BF16, tag=f"x{sc}{dc}",
                          name=f"x{sc}{dc}") for dc in range(4)]
               for sc in range(4)]

        def xp(sc, dc):
            base = OFF_XT + S * dc + 512 * sc
            return wpack[:, base:base + 512]

        nc.sync.dma_start(w_qb[:], wpack[:, OFF_WQ:OFF_WQ + 1024])
        nc.sync.dma_start(w_bc[:], bcol)
        for dc in range(4):
            nc.sync.dma_start(x_t[0][dc][:], xp(0, dc))
        nc.sync.dma_start(w_k[:], wpack[:, OFF_WK:OFF_WK + 1024])
        nc.sync.dma_start(w_misc[:], wpack[:, OFF_ONES:OFF_ONES + 516])
        nc.sync.dma_start(w_wva[:], wpack[:, OFF_WVA:OFF_WVA + 1040])
        for sc in range(1, 4):
            for dc in range(4):
                nc.sync.dma_start(x_t[sc][dc][:], xp(sc, dc))
        nc.sync.dma_start(w_wo[:], wpack[:, OFF_WO:OFF_WO + 1024])

        def wq_d(dc):
            return w_qb[:, 256 * dc:256 * (dc + 1)]

        def wk_d(dc):
            return w_k[:, 256 * dc:256 * (dc + 1)]

        def xd(sc, dc):
            return x_t[sc][dc][:]

        def xd128(st, dc):
            base = 128 * (st % 4)
            return x_t[st // 4][dc][:, base:base + 128]

        def wva_d(dc):
            return w_wva[:, VWS * dc:VWS * (dc + 1)]

        ones64 = w_misc[0:1, 0:64]
        mm_keep = w_misc[:, 128:256]
        vones = w_misc[:, 256:516]

        def wo_p(p):
            return w_wo[:, 512 * p:512 * (p + 1)]

        # ---------- persistent intermediates ----------
        QTs = [[pers.tile([128, 512], BF16, tag=f"QT{p}{sc}",
                          name=f"QT{p}{sc}") for sc in range(4)]
               for p in range(2)]
        KTs = [[pers.tile([128, 512], BF16, tag=f"KT{p}{sc}",
                          name=f"KT{p}{sc}") for sc in range(4)]
               for p in range(2)]
        vaug = [pers.tile([128, VWS], BF16, tag=f"va{st}", name=f"va{st}")
                for st in range(16)]
        OTN = [[pers.tile([128, 512], BF16, tag=f"OTN{p}{qq}",
                          name=f"OTN{p}{qq}") for qq in range(4)]
               for p in range(2)]

        # warm the ACT exp table off the critical path
        warm = dnp.tile([1, 16], BF16, tag="warm", name="warm")
        nc.vector.memset(warm[:], 0.0)
        warm2 = dnp.tile([1, 16], BF16, tag="warm2", name="warm2")
        nc.scalar.activation(warm2[:], warm[:], AF.Exp, scale=SCALE)

        # memset the scores psum buffers once (exp may read lanes no matmul
        # wrote this iteration; stale-but-bounded is fine, uninit is not)
        for _ in range(2):
            ps_s_init = spsum.tile([128, 1024], F32, tag="ps_s", name="ps_s")
            nc.vector.memset(ps_s_init[:], 0.0)

        # ---------- chunk emitters ----------
        def p1a_chunk(pair, qk, sc):
            ps = p1p.tile([128, 512], F32, tag="p1", name="p1a")
            w_d = wq_d if qk == 0 else wk_d
            for dc in range(4):
                nc.tensor.matmul(
                    ps[:], w_d(dc)[:, 128 * pair:128 * (pair + 1)],
                    xd(sc, dc), start=(dc == 0), stop=(dc == 3))
            dst = (QTs if qk == 0 else KTs)[pair][sc]
            nc.vector.tensor_scalar_add(
                dst[:], ps[:], w_bc[:, 2 * pair + qk:2 * pair + qk + 1])

        def p1b_chunk(st):
            ps = p1p.tile([128, VWS], F32, tag="p1", name="p1b")
            for dc in range(4):
                nc.tensor.matmul(ps[:], xd128(st, dc), wva_d(dc),
                                 start=(dc == 0), stop=(dc == 3))
            # ones columns (denominator trick) added during evacuation
            nc.vector.tensor_add(vaug[st][:], ps[:], vones)

        norm_state = {}

        def normA(pair, qq, ps_oo):
            ot = otp.tile([128, 512], F32, tag="ot", name="ot")
            dsl = dnp.tile([1, 1024], F32, tag="dsl", name="dsl")
            for sub in range(2):
                qrows = slice(64 * sub, 64 * sub + 64)
                nc.vector.tensor_copy(ot[qrows, :], ps_oo[sub][0:64, :])
                nc.vector.tensor_copy(dsl[:, 512 * sub:512 * (sub + 1)],
                                      ps_oo[sub][64:65, :])
            dq = dnp.tile([16, 64], F32, tag="dq", name="dq")
            nc.sync.dma_start(dq[:], dsl[:])
            rq = dnp.tile([16, 64], BF16, tag="rq", name="rq")
            with nc.allow_low_precision(reason="bf16 softmax recip"):
                nc.vector.reciprocal(rq[:], dq[:])
            rrow = dnp.tile([1, 1024], BF16, tag="rrow", name="rrow")
            nc.sync.dma_start(rrow[:], rq[:])
            norm_state[(pair, qq)] = (ot, rrow)

        def normB(pair, qq):
            ot, rrow = norm_state.pop((pair, qq))
            for sub in range(2):
                qrows = slice(64 * sub, 64 * sub + 64)
                ps_b = p1p.tile([64, 512], F32, tag="p1", name="ps_b")
                nc.tensor.matmul(ps_b[:], ones64,
                                 rrow[0:1, 512 * sub:512 * (sub + 1)],
                                 start=True, stop=True)
                nc.vector.tensor_mul(OTN[pair][qq][qrows, :],
                                     ot[qrows, :], ps_b[:])

        def p3_chunk(qq, u, on_act=False):
            t = 4 * qq + u
            ps_f = p1p.tile([128, 512], F32, tag="p1", name="ps_f")
            for p in range(2):
                nc.tensor.matmul(ps_f[:], OTN[p][qq][:, 128 * u:128 * (u + 1)],
                                 wo_p(p), start=(p == 0), stop=(p == 1))
            osb = outp.tile([128, 512], F32, tag="osb", name="osb")
            if on_act:
                nc.scalar.copy(osb[:], ps_f[:])
            else:
                nc.vector.tensor_copy(osb[:], ps_f[:])
            nc.sync.dma_start(out[128 * t:128 * (t + 1), :], osb[:])

        def p2_step(pair, qq, fillers, finish_prev=None):
            fillers = list(fillers)
            ps_oo = [opsum.tile([VW, 512], F32, tag=f"ps_o{sub}",
                                name=f"ps_o{sub}") for sub in range(2)]
            nkk = 4 * qq + 4

            def pv(kk, so):
                for sub in range(2):
                    h = 2 * pair + sub
                    nc.tensor.matmul(
                        ps_oo[sub][:, so:512],
                        vaug[kk][:, VW * h:VW * h + VW],
                        pt_k[kk][:, 512 * sub + so:512 * (sub + 1)],
                        start=(kk == 0), stop=(kk == nkk - 1))

            pt_k = {}
            prev = None
            for kk in range(nkk):
                so = max(kk * 128 - qq * 512, 0)
                diag = (kk >= 4 * qq)
                ps_s = spsum.tile([128, 1024], F32, tag="ps_s", name="ps_s")
                if kk == 1 and finish_prev is not None:
                    finish_prev()
                for sub in range(2):
                    qrows = slice(64 * sub, 64 * sub + 64)
                    nc.tensor.matmul(
                        ps_s[:, 512 * sub + so:512 * (sub + 1)],
                        KTs[pair][kk // 4][qrows,
                                           128 * (kk % 4):128 * (kk % 4) + 128],
                        QTs[pair][qq][qrows, so:512],
                        start=True, stop=True)
                pt = ptp.tile([128, 1024], BF16, tag="pt", name="pt")
                pt_k[kk] = pt
                if so == 0:
                    nc.scalar.activation(pt[:], ps_s[:], AF.Exp, scale=SCALE)
                else:
                    pss3 = ps_s[:].rearrange("p (b c) -> p b c", c=512)
                    pt3 = pt[:].rearrange("p (b c) -> p b c", c=512)
                    nc.scalar.activation(pt3[:, :, so:], pss3[:, :, so:],
                                         AF.Exp, scale=SCALE)
                if diag:
                    # zero the strict-upper (k>q) triangle of the 128-wide
                    # diagonal sub-block of each head's probabilities
                    for sub in range(2):
                        blk = slice(512 * sub + so, 512 * sub + so + 128)
                        nc.gpsimd.tensor_mul(pt[:, blk], pt[:, blk], mm_keep)
                # PV lags one block so its exp (and mask) are already done
                if prev is not None:
                    pv(*prev)
                if kk < len(fillers):
                    for f in fillers[kk]:
                        f()
                prev = (kk, so)

            def fin(prev=prev):
                pv(*prev)
                normA(pair, qq, ps_oo)
            return fin

        # ---------- emission schedule ----------
        def c(f, *a):
            return lambda: f(*a)

        p1a_chunk(0, 0, 0)
        p1a_chunk(0, 1, 0)
        p1b_chunk(0)

        fin = p2_step(0, 0, [
            [c(p1b_chunk, 1)], [c(p1b_chunk, 2)], [c(p1b_chunk, 3)],
            [c(p1a_chunk, 0, 0, 1), c(p1a_chunk, 0, 1, 1)],
        ])
        fin = p2_step(0, 1, [
            [c(p1b_chunk, 4)], [c(p1b_chunk, 5)], [c(p1b_chunk, 6)],
            [c(p1b_chunk, 7)], [c(p1a_chunk, 0, 0, 2)],
            [c(p1a_chunk, 0, 1, 2)], [c(normB, 0, 0)], [],
        ], fin)
        fin = p2_step(0, 2, [
            [c(p1b_chunk, 8)], [c(p1b_chunk, 9)], [c(p1b_chunk, 10)],
            [c(p1b_chunk, 11)], [c(p1a_chunk, 0, 0, 3)],
            [c(p1a_chunk, 0, 1, 3)], [c(normB, 0, 1)],
            [c(p1a_chunk, 1, 0, 0)], [c(p1a_chunk, 1, 1, 0)],
            [c(p1a_chunk, 1, 0, 1)], [c(p1a_chunk, 1, 1, 1)], [],
        ], fin)
        fin = p2_step(0, 3, [
            [c(p1b_chunk, 12)], [c(p1b_chunk, 13)], [c(p1b_chunk, 14)],
            [c(p1b_chunk, 15)], [c(p1a_chunk, 1, 0, 2)],
            [c(p1a_chunk, 1, 1, 2)], [c(normB, 0, 2)],
            [c(p1a_chunk, 1, 0, 3)], [c(p1a_chunk, 1, 1, 3)],
            [], [], [], [], [], [], [],
        ], fin)
        fin = p2_step(1, 0, [
            [], [], [c(normB, 0, 3)], [],
        ], fin)
        fin = p2_step(1, 1, [
            [], [], [c(normB, 1, 0)],
            [c(p3_chunk, 0, 0)], [c(p3_chunk, 0, 1)],
            [c(p3_chunk, 0, 2)], [c(p3_chunk, 0, 3)], [],
        ], fin)
        fin = p2_step(1, 2, [
            [], [], [], [c(normB, 1, 1)],
            [c(p3_chunk, 1, 0)], [c(p3_chunk, 1, 1)],
            [c(p3_chunk, 1, 2)], [c(p3_chunk, 1, 3)],
            [], [], [], [],
        ], fin)
        fin = p2_step(1, 3, [
            [], [], [], [c(normB, 1, 2)],
            [c(p3_chunk, 2, 0)], [c(p3_chunk, 2, 1)],
            [c(p3_chunk, 2, 2)], [c(p3_chunk, 2, 3)],
            [], [], [], [], [], [], [], [],
        ], fin)
        fin()
        normB(1, 3)
        for u in range(4):
            p3_chunk(3, u, on_act=(u % 2 == 0))


def make_in_maps(x, w_qkv, b_qkv, w_out, b_out):
    x = np.asarray(x, dtype=np.float32)
    w_qkv = np.asarray(w_qkv, dtype=np.float32)
    b_qkv = np.asarray(b_qkv, dtype=np.float32)
    w_out = np.asarray(w_out, dtype=np.float32)

    wrr = w_qkv.reshape(D, 3, 8, HD)
    br = b_qkv.reshape(3, 8, HD)

    in_maps = []
    for c in range(NCORES):
        b = c // 2
        h0 = 4 * (c % 2)
        xT = np.ascontiguousarray(x[b].T)                       # (512, 2048)
        wq = wrr[:, 0, h0:h0 + 4].reshape(D, 256)
        wk = wrr[:, 1, h0:h0 + 4].reshape(D, 256)
        wv = wrr[:, 2, h0:h0 + 4].reshape(D, 256)
        wva = np.zeros((D, VWS), dtype=np.float32)
        for j in range(HPC):
            wva[:, VW * j:VW * j + HD] = wv[:, HD * j:HD * (j + 1)]
        wo = w_out.reshape(8, HD, D)[h0:h0 + 4].reshape(256, D)

        wpack = np.zeros((128, FTOT), dtype=np.float32)
        for dc in range(4):
            wpack[:, OFF_WQ + 256 * dc:OFF_WQ + 256 * (dc + 1)] = \
                wq[128 * dc:128 * (dc + 1)]
            wpack[:, OFF_WK + 256 * dc:OFF_WK + 256 * (dc + 1)] = \
                wk[128 * dc:128 * (dc + 1)]
            wpack[:, OFF_XT + S * dc:OFF_XT + S * (dc + 1)] = \
                xT[128 * dc:128 * (dc + 1)]
            wpack[:, OFF_WVA + VWS * dc:OFF_WVA + VWS * (dc + 1)] = \
                wva[128 * dc:128 * (dc + 1)]
        for p in range(2):
            wpack[:, OFF_WO + D * p:OFF_WO + D * (p + 1)] = \
                wo[128 * p:128 * (p + 1)]
        wpack[0, OFF_ONES:OFF_ONES + 128] = 1.0
        # keep-mask for the diagonal block: key p kept for query col c iff p<=c
        wpack[:, OFF_MASK:OFF_MASK + 128] = np.triu(np.ones((128, 128)))
        for j in range(HPC):
            wpack[:, OFF_VONES + VW * j + HD] = 1.0

        # per-partition bias columns: [bq p0, bk p0, bq p1, bk p1]
        bcol = np.zeros((128, 4), dtype=np.float32)
        for p in range(2):
            bcol[:, 2 * p + 0] = br[0, h0 + 2 * p:h0 + 2 * p + 2].reshape(128)
            bcol[:, 2 * p + 1] = br[1, h0 + 2 * p:h0 + 2 * p + 2].reshape(128)

        in_maps.append({"wpack": wpack.astype(ml_dtypes.bfloat16),
                        "bcol": bcol})
    return in_maps


_NC_CACHE = None


def get_nc():
    global _NC_CACHE
    if _NC_CACHE is None:
        _NC_CACHE = build_nc()
    return _NC_CACHE


def run_cores(x, w_qkv, b_qkv, w_out, b_out, trace=False, trace_cores=None):
    nc = get_nc()
    in_maps = make_in_maps(x, w_qkv, b_qkv, w_out, b_out)
    br = run_bass_kernel_spmd(
        nc, in_maps, list(range(NCORES)),
        trace=trace, trace_cores=trace_cores)
    return br


def assemble(results, b_out, b_qkv=None, w_out=None):
    b_out = np.asarray(b_out, dtype=np.float32)
    b_eff = b_out
    if b_qkv is not None and w_out is not None:
        # V bias commutes through the softmax average: fold into out bias
        b_v = np.asarray(b_qkv, dtype=np.float32)[2 * D:]
        b_eff = b_out + b_v @ np.asarray(w_out, dtype=np.float32)
    out = np.empty((4, S, D), dtype=np.float32)
    for b in range(4):
        out[b] = results[2 * b]["out"] + results[2 * b + 1]["out"] + b_eff
    return out


def kernel(x, w_qkv, b_qkv, w_out, b_out):
    br = run_cores(x, w_qkv, b_qkv, w_out, b_out, trace=False)
    return assemble(br.results, b_out, b_qkv, w_out)


# revision 34
# speedup vs baseline: 1.0841x; 1.0841x over previous
"""Causal multi-head attention kernel for 8 Trainium2 NeuronCores.

Problem: x(4,2048,512) -> qkv proj -> 8-head causal attention -> out proj.
Sharding: core c handles batch b=c//2, heads 4*(c%2)..4*(c%2)+3.
Each core returns a partial (2048,512) output (its 4 heads' contribution
through w_out); host sums the two cores of each batch and adds the
effective output bias (b_out + b_v @ w_out -- the V bias commutes through
the softmax-weighted average exactly, so it folds into the output bias).

Per-core device algorithm (bf16 matmuls, fp32 psum/softmax), fully
software-pipelined so the PE array never idles (idle gaps drop the PE
p-state from 2.4 GHz to 1.2 GHz for ~3us):

  P1a  QT/KT (128 = 2 heads x 64 hd, 512-query chunks) from
       host-pretransposed xT; Q/K biases added per-partition during the
       PSUM->SBUF evacuation via tensor_scalar_add (no rank-1 matmuls).
  P1b  V stored natural (keys on partitions) with a ones column per head
       so the PV matmul also produces softmax denominators.
  P2   per head pair: S_T = K Q^T (keys on partitions, queries free) in
       fp32 PSUM, causal diag masked by accumulating -1e5*(k>q) via a
       bf16 matmul, exp via ACT (scale=1/8 folded, no max subtraction --
       scores are O(10)), then [V|1]^T @ P^T accumulated in PSUM.
       Denominators: copy -> DMA-gather to 128 lanes -> DVE reciprocal ->
       DMA back to a row -> partition-broadcast via ones(1,64) matmul ->
       elementwise normalize.
  P3   output projection per 128-token chunk, DMA'd out immediately.

  Emission interleaves P1b/P1a(pair1)/normalize/P3 chunks into the P2
  kk-loops as PE fillers, so the exp stream (ACT engine, the secondary
  bottleneck) pipelines back-to-back under continuous PE work.
"""

import sys

import numpy as np

if "/opt/trn_rl_repo" not in sys.path:
    sys.path.insert(0, "/opt/trn_rl_repo")

import ml_dtypes

import concourse.bass as bass
import concourse.mybir as mybir
import concourse.tile as tile
from concourse import bacc
from concourse.bass_utils import run_bass_kernel_spmd

F32 = mybir.dt.float32
BF16 = mybir.dt.bfloat16
AF = mybir.ActivationFunctionType

S = 2048
D = 512
HD = 64
HPC = 4          # heads per core
NCORES = 8
SCALE = 0.125    # 1/sqrt(64)
VW = HD + 1      # 65: V plus ones column
VWS = HPC * VW   # 260

# column offsets inside the packed bf16 (128, FTOT) input
OFF_WQ = 0                      # 4 x 256
OFF_WK = OFF_WQ + 1024          # 4 x 256
OFF_XT = OFF_WK + 1024          # 4 x 2048
OFF_WVA = OFF_XT + 4 * S        # 4 x 260
OFF_WO = OFF_WVA + 4 * VWS      # 2 x 512
OFF_ONES = OFF_WO + 2 * D       # 128 (row 0 ones)
OFF_MASK = OFF_ONES + 128       # (128,128) 0/1 causal keep-mask (k<=q)
OFF_VONES = OFF_MASK + 128      # (128,260) ones-column marks for vaug
FTOT = OFF_VONES + VWS


def build_nc():
    nc = bacc.Bacc("TRN2", target_bir_lowering=False, debug=False)

    wpack = nc.dram_tensor("wpack", [128, FTOT], BF16,
                           kind="ExternalInput").ap()
    bcol = nc.dram_tensor("bcol", [128, 4], F32, kind="ExternalInput").ap()
    out = nc.dram_tensor("out", [S, D], F32, kind="ExternalOutput").ap()

    with tile.TileContext(nc) as tc:
        _build_kernel(tc, wpack, bcol, out)
    nc.compile()
    return nc


def _build_kernel(tc, wpack, bcol, out):
    nc = tc.nc
    from contextlib import ExitStack

    ctx = ExitStack()
    with ctx:
        pers = ctx.enter_context(tc.tile_pool(name="pers", bufs=1))
        spsum = ctx.enter_context(
            tc.tile_pool(name="spsum", bufs=2, space="PSUM"))   # scores
        opsum = ctx.enter_context(
            tc.tile_pool(name="opsum", bufs=1, space="PSUM"))   # PV accum
        p1p = ctx.enter_context(
            tc.tile_pool(name="p1p", bufs=2, space="PSUM"))     # P1/P3/bcast
        ptp = ctx.enter_context(tc.tile_pool(name="ptp", bufs=5))
        otp = ctx.enter_context(tc.tile_pool(name="otp", bufs=4))
        outp = ctx.enter_context(tc.tile_pool(name="outp", bufs=3))
        dnp = ctx.enter_context(tc.tile_pool(name="dnp", bufs=3))

        # ---------- input tiles + DMAs, compute-start order ----------
        w_qb = pers.tile([128, 1024], BF16, tag="w_qb", name="w_qb")
        w_bc = pers.tile([128, 4], F32, tag="w_bc", name="w_bc")
        w_k = pers.tile([128, 1024], BF16, tag="w_k", name="w_k")
        w_misc = pers.tile([128, 516], BF16, tag="w_misc", name="w_misc")
        w_wva = pers.tile([128, 1040], BF16, tag="w_wva", name="w_wva")
        w_wo = pers.tile([128, 1024], BF16, tag="w_wo", name="w_wo")
        x_t = [[pers.tile([128, 512], BF16, tag=f"x{sc}{dc}",
                          name=f"x{sc}{dc}") for dc in range(4)]
               for sc in range(4)]

        def xp(sc, dc):
            base = OFF_XT + S * dc + 512 * sc
            return wpack[:, base:base + 512]

        nc.sync.dma_start(w_qb[:], wpack[:, OFF_WQ:OFF_WQ + 1024])
        nc.sync.dma_start(w_bc[:], bcol)
        for dc in range(4):
            nc.sync.dma_start(x_t[0][dc][:], xp(0, dc))
        nc.sync.dma_start(w_k[:], wpack[:, OFF_WK:OFF_WK + 1024])
        nc.sync.dma_start(w_misc[:], wpack[:, OFF_ONES:OFF_ONES + 516])
        nc.sync.dma_start(w_wva[:], wpack[:, OFF_WVA:OFF_WVA + 1040])
        for sc in range(1, 4):
            for dc in range(4):
                nc.sync.dma_start(x_t[sc][dc][:], xp(sc, dc))
        nc.sync.dma_start(w_wo[:], wpack[:, OFF_WO:OFF_WO + 1024])

        def wq_d(dc):
            return w_qb[:, 256 * dc:256 * (dc + 1)]

        def wk_d(dc):
            return w_k[:, 256 * dc:256 * (dc + 1)]

        def xd(sc, dc):
            return x_t[sc][dc][:]

        def xd128(st, dc):
            base = 128 * (st % 4)
            return x_t[st // 4][dc][:, base:base + 128]

        def wva_d(dc):
            return w_wva[:, VWS * dc:VWS * (dc + 1)]

        ones64 = w_misc[0:1, 0:64]
        mm_keep = w_misc[:, 128:256]
        vones = w_misc[:, 256:516]

        def wo_p(p):
            return w_wo[:, 512 * p:512 * (p + 1)]

        # ---------- persistent intermediates ----------
        QTs = [[pers.tile([128, 512], BF16, tag=f"QT{p}{sc}",
                          name=f"QT{p}{sc}") for sc in range(4)]
               for p in range(2)]
        KTs = [[pers.tile([128, 512], BF16, tag=f"KT{p}{sc}",
                          name=f"KT{p}{sc}") for sc in range(4)]
               for p in range(2)]
        vaug = [pers.tile([128, VWS], BF16, tag=f"va{st}", name=f"va{st}")
                for st in range(16)]
        OTN = [[pers.tile([128, 512], BF16, tag=f"OTN{p}{qq}",
                          name=f"OTN{p}{qq}") for qq in range(4)]
               for p in range(2)]

        # warm the ACT exp table off the critical path
        warm = dnp.tile([1, 16], BF16, tag="warm", name="warm")
        nc.vector.memset(warm[:], 0.0)
        warm2 = dnp.tile([1, 16], BF16, tag="warm2", name="warm2")
        nc.scalar.activation(warm2[:], warm[:], AF.Exp, scale=SCALE)

        # memset the scores psum buffers once (exp may read lanes no matmul
        # wrote this iteration; stale-but-bounded is fine, uninit is not)
        for _ in range(2):
            ps_s_init = spsum.tile([128, 1024], F32, tag="ps_s", name="ps_s")
            nc.vector.memset(ps_s_init[:], 0.0)

        # ---------- chunk emitters ----------
        def p1a_chunk(pair, qk, sc):
            ps = p1p.tile([128, 512], F32, tag="p1", name="p1a")
            w_d = wq_d if qk == 0 else wk_d
            for dc in range(4):
                nc.tensor.matmul(
                    ps[:], w_d(dc)[:, 128 * pair:128 * (pair + 1)],
                    xd(sc, dc), start=(dc == 0), stop=(dc == 3))
            dst = (QTs if qk == 0 else KTs)[pair][sc]
            nc.vector.tensor_scalar_add(
                dst[:], ps[:], w_bc[:, 2 * pair + qk:2 * pair + qk + 1])

        def p1b_chunk(st):
            ps = p1p.tile([128, VWS], F32, tag="p1", name="p1b")
            for dc in range(4):
                nc.tensor.matmul(ps[:], xd128(st, dc), wva_d(dc),
                                 start=(dc == 0), stop=(dc == 3))
            # ones columns (denominator trick) added during evacuation
            nc.vector.tensor_add(vaug[st][:], ps[:], vones)

        norm_state = {}

        def normA(pair, qq, ps_oo):
            ot = otp.tile([128, 512], F32, tag="ot", name="ot")
            dsl = dnp.tile([1, 1024], F32, tag="dsl", name="dsl")
            for sub in range(2):
                qrows = slice(64 * sub, 64 * sub + 64)
                nc.vector.tensor_copy(ot[qrows, :], ps_oo[sub][0:64, :])
                nc.vector.tensor_copy(dsl[:, 512 * sub:512 * (sub + 1)],
                                      ps_oo[sub][64:65, :])
            dq = dnp.tile([16, 64], F32, tag="dq", name="dq")
            nc.sync.dma_start(dq[:], dsl[:])
            rq = dnp.tile([16, 64], BF16, tag="rq", name="rq")
            with nc.allow_low_precision(reason="bf16 softmax recip"):
                nc.vector.reciprocal(rq[:], dq[:])
            rrow = dnp.tile([1, 1024], BF16, tag="rrow", name="rrow")
            nc.sync.dma_start(rrow[:], rq[:])
            norm_state[(pair, qq)] = (ot, rrow)

        def normB(pair, qq):
            ot, rrow = norm_state.pop((pair, qq))
            for sub in range(2):
                qrows = slice(64 * sub, 64 * sub + 64)
                ps_b = p1p.tile([64, 512], F32, tag="p1", name="ps_b")
                nc.tensor.matmul(ps_b[:], ones64,
                                 rrow[0:1, 512 * sub:512 * (sub + 1)],
                                 start=True, stop=True)
                nc.vector.tensor_mul(OTN[pair][qq][qrows, :],
                                     ot[qrows, :], ps_b[:])

        def p3_chunk(qq, u, on_act=False):
            t = 4 * qq + u
            ps_f = p1p.tile([128, 512], F32, tag="p1", name="ps_f")
            for p in range(2):
                nc.tensor.matmul(ps_f[:], OTN[p][qq][:, 128 * u:128 * (u + 1)],
                                 wo_p(p), start=(p == 0), stop=(p == 1))
            osb = outp.tile([128, 512], F32, tag="osb", name="osb")
            if on_act:
                nc.scalar.copy(osb[:], ps_f[:])
            else:
                nc.vector.tensor_copy(osb[:], ps_f[:])
            nc.sync.dma_start(out[128 * t:128 * (t + 1), :], osb[:])

        def p2_step(pair, qq, fillers, finish_prev=None):
            fillers = list(fillers)
            ps_oo = [opsum.tile([VW, 512], F32, tag=f"ps_o{sub}",
                                name=f"ps_o{sub}") for sub in range(2)]
            nkk = 4 * qq + 4

            def pv(kk, so):
                for sub in range(2):
                    h = 2 * pair + sub
                    nc.tensor.matmul(
                        ps_oo[sub][:, so:512],
                        vaug[kk][:, VW * h:VW * h + VW],
                        pt_k[kk][:, 512 * sub + so:512 * (sub + 1)],
                        start=(kk == 0), stop=(kk == nkk - 1))

            pt_k = {}
            pend = []
            for kk in range(nkk):
                so = max(kk * 128 - qq * 512, 0)
                diag = (kk >= 4 * qq)
                ps_s = spsum.tile([128, 1024], F32, tag="ps_s", name="ps_s")
                if kk == 1 and finish_prev is not None:
                    finish_prev()
                for sub in range(2):
                    qrows = slice(64 * sub, 64 * sub + 64)
                    nc.tensor.matmul(
                        ps_s[:, 512 * sub + so:512 * (sub + 1)],
                        KTs[pair][kk // 4][qrows,
                                           128 * (kk % 4):128 * (kk % 4) + 128],
                        QTs[pair][qq][qrows, so:512],
                        start=True, stop=True)
                pt = ptp.tile([128, 1024], BF16, tag="pt", name="pt")
                pt_k[kk] = pt
                if so == 0:
                    nc.scalar.activation(pt[:], ps_s[:], AF.Exp, scale=SCALE)
                else:
                    pss3 = ps_s[:].rearrange("p (b c) -> p b c", c=512)
                    pt3 = pt[:].rearrange("p (b c) -> p b c", c=512)
                    nc.scalar.activation(pt3[:, :, so:], pss3[:, :, so:],
                                         AF.Exp, scale=SCALE)
                if diag:
                    # zero the strict-upper (k>q) triangle of the 128-wide
                    # diagonal sub-block of each head's probabilities
                    for sub in range(2):
                        blk = slice(512 * sub + so, 512 * sub + so + 128)
                        nc.vector.tensor_mul(pt[:, blk], pt[:, blk], mm_keep)
                # PV lags two blocks so its exp (and mask) are already done
                pend.append((kk, so))
                if len(pend) > 2:
                    pv(*pend.pop(0))
                if kk < len(fillers):
                    for f in fillers[kk]:
                        f()

            def fin(pend=pend):
                for pr in pend:
                    pv(*pr)
                normA(pair, qq, ps_oo)
            return fin

        # ---------- emission schedule ----------
        def c(f, *a):
            return lambda: f(*a)

        p1a_chunk(0, 0, 0)
        p1a_chunk(0, 1, 0)
        p1b_chunk(0)

        fin = p2_step(0, 0, [
            [c(p1b_chunk, 1)], [c(p1b_chunk, 2)], [c(p1b_chunk, 3)],
            [c(p1a_chunk, 0, 0, 1), c(p1a_chunk, 0, 1, 1)],
        ])
        fin = p2_step(0, 1, [
            [c(p1b_chunk, 4)], [c(p1b_chunk, 5)], [c(p1b_chunk, 6)],
            [c(p1b_chunk, 7)], [c(p1a_chunk, 0, 0, 2)],
            [c(p1a_chunk, 0, 1, 2)], [c(normB, 0, 0)], [],
        ], fin)
        fin = p2_step(0, 2, [
            [c(p1b_chunk, 8)], [c(p1b_chunk, 9)], [c(p1b_chunk, 10)],
            [c(p1b_chunk, 11)], [c(p1a_chunk, 0, 0, 3)],
            [c(p1a_chunk, 0, 1, 3)], [c(normB, 0, 1)],
            [c(p1a_chunk, 1, 0, 0)], [c(p1a_chunk, 1, 1, 0)],
            [c(p1a_chunk, 1, 0, 1)], [c(p1a_chunk, 1, 1, 1)], [],
        ], fin)
        fin = p2_step(0, 3, [
            [c(p1b_chunk, 12)], [c(p1b_chunk, 13)], [c(p1b_chunk, 14)],
            [c(p1b_chunk, 15)], [c(p1a_chunk, 1, 0, 2)],
            [c(p1a_chunk, 1, 1, 2)], [c(normB, 0, 2)],
            [c(p1a_chunk, 1, 0, 3)], [c(p1a_chunk, 1, 1, 3)],
            [], [], [], [], [], [], [],
        ], fin)
        fin = p2_step(1, 0, [
            [], [], [c(normB, 0, 3)], [],
        ], fin)
        fin = p2_step(1, 1, [
            [], [], [c(normB, 1, 0)],
            [c(p3_chunk, 0, 0)], [c(p3_chunk, 0, 1)],
            [c(p3_chunk, 0, 2)], [c(p3_chunk, 0, 3)], [],
        ], fin)
        fin = p2_step(1, 2, [
            [], [], [], [c(normB, 1, 1)],
            [c(p3_chunk, 1, 0)], [c(p3_chunk, 1, 1)],
            [c(p3_chunk, 1, 2)], [c(p3_chunk, 1, 3)],
            [], [], [], [],
        ], fin)
        fin = p2_step(1, 3, [
            [], [], [], [c(normB, 1, 2)],
            [c(p3_chunk, 2, 0)], [c(p3_chunk, 2, 1)],
            [], [], [], [], [], [], [], [], [], [],
        ], fin)
        fin()
        # fill the final normalize chain's latency with the remaining
        # projection chunks of qq=2
        p3_chunk(2, 2)
        p3_chunk(2, 3)
        normB(1, 3)
        for u in range(4):
            p3_chunk(3, u, on_act=(u % 2 == 0))


def make_in_maps(x, w_qkv, b_qkv, w_out, b_out):
    x = np.asarray(x, dtype=np.float32)
    w_qkv = np.asarray(w_qkv, dtype=np.float32)
    b_qkv = np.asarray(b_qkv, dtype=np.float32)
    w_out = np.asarray(w_out, dtype=np.float32)

    wrr = w_qkv.reshape(D, 3, 8, HD)
    br = b_qkv.reshape(3, 8, HD)

    in_maps = []
    for c in range(NCORES):
        b = c // 2
        h0 = 4 * (c % 2)
        xT = np.ascontiguousarray(x[b].T)                       # (512, 2048)
        wq = wrr[:, 0, h0:h0 + 4].reshape(D, 256)
        wk = wrr[:, 1, h0:h0 + 4].reshape(D, 256)
        wv = wrr[:, 2, h0:h0 + 4].reshape(D, 256)
        wva = np.zeros((D, VWS), dtype=np.float32)
        for j in range(HPC):
            wva[:, VW * j:VW * j + HD] = wv[:, HD * j:HD * (j + 1)]
        wo = w_out.reshape(8, HD, D)[h0:h0 + 4].reshape(256, D)

        wpack = np.zeros((128, FTOT), dtype=np.float32)
        for dc in range(4):
            wpack[:, OFF_WQ + 256 * dc:OFF_WQ + 256 * (dc + 1)] = \
                wq[128 * dc:128 * (dc + 1)]
            wpack[:, OFF_WK + 256 * dc:OFF_WK + 256 * (dc + 1)] = \
                wk[128 * dc:128 * (dc + 1)]
            wpack[:, OFF_XT + S * dc:OFF_XT + S * (dc + 1)] = \
                xT[128 * dc:128 * (dc + 1)]
            wpack[:, OFF_WVA + VWS * dc:OFF_WVA + VWS * (dc + 1)] = \
                wva[128 * dc:128 * (dc + 1)]
        for p in range(2):
            wpack[:, OFF_WO + D * p:OFF_WO + D * (p + 1)] = \
                wo[128 * p:128 * (p + 1)]
        wpack[0, OFF_ONES:OFF_ONES + 128] = 1.0
        # keep-mask for the diagonal block: key p kept for query col c iff p<=c
        wpack[:, OFF_MASK:OFF_MASK + 128] = np.triu(np.ones((128, 128)))
        for j in range(HPC):
            wpack[:, OFF_VONES + VW * j + HD] = 1.0

        # per-partition bias columns: [bq p0, bk p0, bq p1, bk p1]
        bcol = np.zeros((128, 4), dtype=np.float32)
        for p in range(2):
            bcol[:, 2 * p + 0] = br[0, h0 + 2 * p:h0 + 2 * p + 2].reshape(128)
            bcol[:, 2 * p + 1] = br[1, h0 + 2 * p:h0 + 2 * p + 2].reshape(128)

        in_maps.append({"wpack": wpack.astype(ml_dtypes.bfloat16),
                        "bcol": bcol})
    return in_maps


_NC_CACHE = None


def get_nc():
    global _NC_CACHE
    if _NC_CACHE is None:
        _NC_CACHE = build_nc()
    return _NC_CACHE


def run_cores(x, w_qkv, b_qkv, w_out, b_out, trace=False, trace_cores=None):
    nc = get_nc()
    in_maps = make_in_maps(x, w_qkv, b_qkv, w_out, b_out)
    br = run_bass_kernel_spmd(
        nc, in_maps, list(range(NCORES)),
        trace=trace, trace_cores=trace_cores)
    return br


def assemble(results, b_out, b_qkv=None, w_out=None):
    b_out = np.asarray(b_out, dtype=np.float32)
    b_eff = b_out
    if b_qkv is not None and w_out is not None:
        # V bias commutes through the softmax average: fold into out bias
        b_v = np.asarray(b_qkv, dtype=np.float32)[2 * D:]
        b_eff = b_out + b_v @ np.asarray(w_out, dtype=np.float32)
    out = np.empty((4, S, D), dtype=np.float32)
    for b in range(4):
        out[b] = results[2 * b]["out"] + results[2 * b + 1]["out"] + b_eff
    return out


def kernel(x, w_qkv, b_qkv, w_out, b_out):
    br = run_cores(x, w_qkv, b_qkv, w_out, b_out, trace=False)
    return assemble(br.results, b_out, b_qkv, w_out)
